# revision 48
# baseline (speedup 1.0000x reference)
"""Trainium2 Bass kernel for nn_DFlashSelfAttention (block-sparse GQA attention).

Self-contained: builds the Bass module once, shards inputs over 8 NeuronCores
(sequence-parallel), runs via run_bass_kernel_spmd, reassembles full output.
"""

import sys as _sys
for _p in ("/opt/trn_rl_repo",):
    if _p not in _sys.path:
        _sys.path.insert(0, _p)

"""Bass/Tile kernel for DFlashSelfAttention (block-diagonal causal attention).

Sharding: sequence-parallel over L (2048 -> 8 cores x 256 positions).
Attention is block-diagonal with BLOCK=16, so positions never interact
across 16-blocks; a 256-position slice (16 blocks) is fully independent.

Per-core pipeline (T = 512 rows = 2 batches x 256 positions):
  phase A: Q = X @ Wq, one pass over Wq, all 8 psum banks.
  phase B: KV = X @ Wkv into banks freed by Q->SBUF drains.
  per 128-token chunk: fp16 RMS-norm+RoPE (rstd = exp(-ln(v)/2) keeps the
    ACT engine on one table set), PE pair-transposes of Q^T/K^T, then
    GROUP-BATCHED attention: per kv-head group g of 4 query heads one
    [128,512] psum tile holds mask+S for (head, qtok); one ACT exp; column
    sums via GPSIMD partition_all_reduce; one AV matmul lands at psum
    partitions (g%2)*64; normalization = per-pair DVE reciprocal + one DVE
    multiply straight from PSUM into A^T.
  emit: Y^T = Wo^T @ A^T per 256-token half, interleaved with the chunk-2/3
    attention chains; fp16 DRAM [4096, 512]; host transposes back.

All matmul operands fp16 (1 cy/row); psum accumulation f32; softmax in f32
via ACT exp. RMS-norm weights and the sqrt(1/8) attention scale are folded
into host-precomputed rope tables.
"""

import ml_dtypes
import numpy as np

import concourse.bass as bass
import concourse.mybir as mybir
import concourse.tile as tile
from concourse import bacc
from concourse.bass_isa import ReduceOp
from concourse.masks import make_identity

F32 = mybir.dt.float32
F16 = mybir.dt.float16

P = 128
HID = 4096
KO = HID // P          # 32 k-chunks over hidden
T = 512                # rows per core: 2 batches x 256 positions
NM = T // P            # 4 t-chunks
NH = 16
NKV = 4
HD = 64
H2 = HD // 2
QD = NH * HD           # 1024
KVD = 2 * NKV * HD     # 512 (k 256 | v 256)
EPS = 1e-6

# Q-head permutation: position p holds original head PERM[p]. Even positions
# carry heads of even kv parity groups; transposed pair-tiles then expose each
# kv group's 4 heads as one contiguous [64, 4, 128] moving operand.
PERM = [0, 4, 1, 5, 2, 6, 3, 7, 8, 12, 9, 13, 10, 14, 11, 15]


def _pin_act_tables():
    """Make every activation resolve to the natural_log_exp_and_others set so
    the act-table pass emits one load instead of ping-ponging between the
    exp-only and ln-only tables. Indices (act_func_set_id) are preserved."""
    import concourse.bacc as _bacc_mod
    from concourse import hw_specs as _hw
    real = _hw.get_activation_tables

    def pinned(arch):
        t = real(arch)
        keep = "natural_log_exp_and_others"
        if keep not in t:
            return t
        return {nm: (fns if nm == keep else set()) for nm, fns in t.items()}

    _bacc_mod.get_activation_tables = pinned
    return _bacc_mod, real


def build_nc(name="dfa"):
    _bacc_mod, _real_gat = _pin_act_tables()
    try:
        return _build_nc_inner(name)
    finally:
        _bacc_mod.get_activation_tables = _real_gat


def _build_nc_inner(name="dfa"):
    nc = bacc.Bacc(None, target_bir_lowering=False, name=name)

    xt = nc.dram_tensor("xt", [HID, T], F16, kind="ExternalInput")
    wq = nc.dram_tensor("wq", [HID, QD], F16, kind="ExternalInput")
    wkv = nc.dram_tensor("wkv", [HID, KVD], F16, kind="ExternalInput")
    wo = nc.dram_tensor("wo", [P, 32, 8, P], F16, kind="ExternalInput")
    tabs = nc.dram_tensor("tabs", [T, 4 * HD], F16, kind="ExternalInput")
    mask4 = nc.dram_tensor("mask4", [P, 4 * P], F16, kind="ExternalInput")
    yt = nc.dram_tensor("yt", [HID, T], F16, kind="ExternalOutput")

    xt_r = xt.rearrange("(ko p) t -> p ko t", p=P)
    wq_r = wq.rearrange("(kb p) n -> p kb n", p=P)
    wkv_r = wkv.rearrange("(kb p) n -> p kb n", p=P)
    tabs_r = tabs.rearrange("(m p) d -> p m d", p=P)
    yt_r = yt.rearrange("(mo p) t -> p mo t", p=P)

    from contextlib import ExitStack
    with tile.TileContext(nc) as tc, ExitStack() as ctx:
        consts = ctx.enter_context(tc.tile_pool(name="consts", bufs=1))
        xt_pool = ctx.enter_context(tc.tile_pool(name="xt", bufs=1))
        wstream = ctx.enter_context(tc.tile_pool(name="wstream", bufs=4))
        acts = ctx.enter_context(tc.tile_pool(name="acts", bufs=1))
        rope_tmp = ctx.enter_context(tc.tile_pool(name="rope_tmp", bufs=1))
        attn_tmp = ctx.enter_context(tc.tile_pool(name="attn_tmp", bufs=2))
        ystage = ctx.enter_context(tc.tile_pool(name="ystage", bufs=2))
        pp = ctx.enter_context(tc.tile_pool(name="pp", bufs=1, space="PSUM"))

        def ptile(shape, bank, name, dtype=F32):
            tot = 512 if dtype == F32 else 1024
            pad = list(shape)
            pad[-1] = max(1, tot // int(np.prod(shape[1:-1])))
            return pp.tile(shape, dtype, tag=f"b{bank}", name=name,
                           padded_shape=pad)

        # ---- leading DMAs: wq/xt fine-grained, then the rest ----
        xt_sb = xt_pool.tile([P, KO, T], F16)
        wq_fine = []
        for k in range(4):
            nc.sync.dma_start(xt_sb[:, k:k + 1, :], xt_r[:, k:k + 1, :])
            wqt = wstream.tile([P, 1, QD], F16, tag="wq1", bufs=4,
                               name=f"wq_f{k}")
            nc.sync.dma_start(wqt[:], wq_r[:, k:k + 1, :])
            wq_fine.append(wqt)

        # constants ride right after the first fine loads
        ident = consts.tile([P, P], F16)
        make_identity(nc, ident)
        eps_t = consts.tile([P, 1], F32)
        nc.vector.memset(eps_t, EPS)

        # wq coarse batches (bufs=4 so FIFO WAR stalls never starve phase A);
        # mask/tabs/wkv woven in behind the first batches
        wq_coarse = []
        wkv_tiles = []
        mask_sb = consts.tile([P, 4, P], F16)
        tabs_sb = consts.tile([P, NM, 4, HD], F16)

        def wkv_load(kb):
            wkvt = wstream.tile([P, 4, KVD], F16, tag="wkv", bufs=8,
                                name=f"wkv_k{kb}")
            nc.sync.dma_start(wkvt[:], wkv_r[:, kb * 4:(kb + 1) * 4, :])
            wkv_tiles.append(wkvt)

        for kb in range(1, 8):
            wqt = wstream.tile([P, 4, QD], F16, tag="wq4", bufs=4,
                               name=f"wq_b{kb}")
            nc.sync.dma_start(wqt[:], wq_r[:, kb * 4:(kb + 1) * 4, :])
            wq_coarse.append(wqt)
            nc.sync.dma_start(xt_sb[:, kb * 4:(kb + 1) * 4, :],
                              xt_r[:, kb * 4:(kb + 1) * 4, :])
            if kb == 2:
                nc.sync.dma_start(
                    mask_sb[:], mask4.rearrange("p (f q) -> p f q", q=P))
                nc.sync.dma_start(
                    tabs_sb[:], tabs_r.rearrange("p m (f d) -> p m f d", d=HD))
            if kb >= 3:
                wkv_load(kb - 3)
        for kb in range(5, 8):
            wkv_load(kb)

        # PE warmup: anchor the p-state ramp while the first DMAs land
        warm_src = consts.tile([P, 512], F16)
        nc.vector.memset(warm_src, 0.0)
        for w in range(3):
            wps = ptile([P, 512], 0, f"warm{w}")
            nc.tensor.matmul(wps[:], ident[:], warm_src[:],
                             start=True, stop=True)

        # ---- phase A: Q = X @ Wq (single pass, all 8 banks) ----
        ps_a = [[ptile([P, 512], 2 * m + s, f"psa{m}_{s}")
                 for s in range(2)] for m in range(NM)]

        def a_matmuls(k, wq_ap):
            for m in range(NM):
                for s in range(2):
                    nc.tensor.matmul(
                        ps_a[m][s][:], xt_sb[:, k, m * P:(m + 1) * P],
                        wq_ap[:, s * 512:(s + 1) * 512],
                        start=(k == 0), stop=(k == KO - 1))

        for k in range(4):
            a_matmuls(k, wq_fine[k][:, 0, :])
        for kb in range(1, 8):
            for kk in range(4):
                a_matmuls(kb * 4 + kk, wq_coarse[kb - 1][:, kk, :])

        # ---- Q drains (DVE/ACT split) + phase B + KV drains ----
        qn = [acts.tile([P, NH, HD], F16, tag=f"qn{m}", name=f"qn{m}")
              for m in range(NM)]

        def qn_copy(m):
            nc.vector.tensor_copy(
                qn[m][:, 0:8, :],
                ps_a[m][0][:].rearrange("p (h d) -> p h d", d=HD))
            nc.scalar.copy(
                qn[m][:, 8:16, :],
                ps_a[m][1][:].rearrange("p (h d) -> p h d", d=HD))

        ps_b = []
        kn = []
        v_sb = []

        def b_phase(m):
            pb = ptile([P, KVD], 6 if m == 3 else m, f"psb{m}")
            ps_b.append(pb)
            for kb in range(8):
                for kk in range(4):
                    k = kb * 4 + kk
                    nc.tensor.matmul(
                        pb[:], xt_sb[:, k, m * P:(m + 1) * P],
                        wkv_tiles[kb][:, kk, :],
                        start=(k == 0), stop=(k == KO - 1))

        def kv_drain(m):
            knt = acts.tile([P, NKV, HD], F16, tag=f"kn{m}", name=f"kn{m}")
            nc.vector.tensor_copy(
                knt[:], ps_b[m][:, 0:256].rearrange("p (h d) -> p h d", d=HD))
            kn.append(knt)
            vt = acts.tile([P, 256], F16, tag=f"v{m}", name=f"v{m}")
            nc.scalar.copy(vt[:], ps_b[m][:, 256:512])
            v_sb.append(vt)

        qn_copy(0)
        qn_copy(1)
        b_phase(0)
        kv_drain(0)
        qn_copy(2)
        qn_copy(3)
        b_phase(1)
        kv_drain(1)
        b_phase(2)
        kv_drain(2)

        # ---- rope helpers (fp16 math; rstd via ln/exp on ACT) ----
        def stats(src, nh, m, tag):
            sq = rope_tmp.tile([P, nh, HD], F16, tag=f"sq{nh}", bufs=1,
                               name=f"sq_{tag}")
            nc.vector.tensor_mul(sq[:], src[:], src[:])
            ssq = rope_tmp.tile([P, nh], F32, tag=f"ssq{nh}", bufs=2,
                                name=f"ssq_{tag}")
            nc.vector.reduce_sum(ssq[:], sq[:], axis=mybir.AxisListType.X)
            lnv = rope_tmp.tile([P, nh], F32, tag=f"lnv{nh}", bufs=2,
                                name=f"lnv_{tag}")
            nc.scalar.activation(lnv[:], ssq[:],
                                 mybir.ActivationFunctionType.Ln,
                                 bias=eps_t[:], scale=1.0 / HD)
            rstd = rope_tmp.tile([P, nh], F16, tag=f"rstd{nh}", bufs=2,
                                 name=f"rstd_{tag}")
            nc.scalar.activation(rstd[:], lnv[:],
                                 mybir.ActivationFunctionType.Exp, scale=-0.5)
            return rstd

        def rope_apply(src, rstd, m, nh, cf, sf, tag):
            qn2 = rope_tmp.tile([P, nh, HD], F16, tag=f"qn2_{nh}", bufs=2,
                                name=f"qn2_{tag}")
            nc.vector.tensor_mul(qn2[:], src[:],
                                 rstd[:, :, None].to_broadcast((P, nh, HD)))
            ctab = tabs_sb[:, m, cf, :]
            stab = tabs_sb[:, m, sf, :]
            o1 = rope_tmp.tile([P, nh, HD], F16, tag=f"o1_{nh}", bufs=1,
                               name=f"o1_{tag}")
            nc.vector.tensor_mul(o1[:], qn2[:],
                                 ctab[:, None, :].to_broadcast((P, nh, HD)))
            o2 = rope_tmp.tile([P, nh, HD], F16, tag=f"o2_{nh}", bufs=1,
                               name=f"o2_{tag}")
            nc.vector.tensor_mul(
                o2[:, :, 0:H2], qn2[:, :, H2:HD],
                stab[:, None, 0:H2].to_broadcast((P, nh, H2)))
            nc.vector.tensor_mul(
                o2[:, :, H2:HD], qn2[:, :, 0:H2],
                stab[:, None, H2:HD].to_broadcast((P, nh, H2)))
            out = rope_tmp.tile([P, nh * HD], F16, tag=f"ro_{nh}", bufs=2,
                                name=f"ro_{tag}")
            nc.vector.tensor_add(out[:], o1[:].rearrange("p h d -> p (h d)"),
                                 o2[:].rearrange("p h d -> p (h d)"))
            return out

        QRT_BANK = {0: 5, 1: 7, 2: 5, 3: 7}
        krT = []
        qrT = []

        def k_pre(m):
            rstd_k = stats(kn[m], NKV, m, f"k{m}")
            kr = rope_apply(kn[m], rstd_k, m, NKV, 2, 3, f"k{m}")
            kps = ptile([P, 2, P], 4, f"krT_ps{m}", F16)
            for j in range(2):
                nc.tensor.matmul(kps[:, j, :], kr[:, j * P:(j + 1) * P],
                                 ident[:], is_transpose=True)
            kt = acts.tile([P, 2, P], F16, tag=f"krT{m}", name=f"krT{m}")
            nc.vector.tensor_copy(kt[:], kps[:])
            krT.append(kt)

        def q_pre(m):
            rstd_q = stats(qn[m], NH, m, f"q{m}")
            qr = rope_apply(qn[m], rstd_q, m, NH, 0, 1, f"q{m}")
            qps = ptile([P, 8, P], QRT_BANK[m], f"qrT_ps{m}", F16)
            for j in range(8):
                nc.tensor.matmul(qps[:, j, :],
                                 qr[:, 2 * j * HD:(2 * j + 2) * HD],
                                 ident[:], is_transpose=True)
            qt = acts.tile([P, 8, P], F16, tag=f"qrT{m}", name=f"qrT{m}")
            nc.vector.tensor_copy(qt[:], qps[:])
            qrT.append(qt)

        # A^T accumulator: partition (g%2)*64+d, col c=(g//2)*4+i, token t.
        at_sb = acts.tile([P, 8, T], F16, tag="at")

        # ---- group-batched attention ----
        attn_state = {}

        def attn_core(m):
            sums = attn_tmp.tile([P, 4, 512], F32, tag="sums", bufs=1,
                                 name=f"sums{m}")
            rcp = attn_tmp.tile([P, 4, 512], F32, tag="rcp", bufs=1,
                                name=f"rcp{m}")
            o_ps = {}
            attn_state[m] = (rcp, o_ps)
            for g in range(4):
                base = (g % 2) * HD
                gp = g // 2
                s_ps = ptile([P, 512], g % 2, f"s{m}_{g}")
                nc.tensor.matmul(s_ps[:], ident[:], mask_sb[:],
                                 start=True, stop=False)
                nc.tensor.matmul(s_ps[:],
                                 krT[m][base:base + HD, gp, :],
                                 qrT[m][base:base + HD, 4 * gp:4 * gp + 4, :],
                                 start=False, stop=True)
                est = attn_tmp.tile([P, 4, P], F16, tag="est", bufs=3,
                                    name=f"est{m}_{g}")
                nc.scalar.activation(est[:], s_ps[:],
                                     mybir.ActivationFunctionType.Exp)
                nc.gpsimd.partition_all_reduce(sums[:, g, :], est[:],
                                               channels=P,
                                               reduce_op=ReduceOp.add)
                if gp not in o_ps:
                    o_ps[gp] = ptile([P, 512], 2 + gp, f"o{m}_{gp}")
                nc.tensor.matmul(o_ps[gp][base:base + HD, :],
                                 v_sb[m][:, g * HD:(g + 1) * HD], est[:],
                                 start=True, stop=True)
                if g % 2 == 1:
                    # per-pair reciprocal: never blocks the DVE queue long
                    nc.vector.reciprocal(rcp[:, g - 1:g + 1, :],
                                         sums[:, g - 1:g + 1, :])

        def attn_norm(m, spread=False):
            rcp, o_ps = attn_state[m]
            for g in range(4):
                base = (g % 2) * HD
                gp = g // 2
                at_dst = at_sb[base:base + HD, 4 * gp:4 * gp + 4,
                               m * P:(m + 1) * P]
                rcp_src = rcp[base:base + HD, g, :].rearrange(
                    "p (i t) -> p i t", t=P)
                o_src = o_ps[gp][base:base + HD, :].rearrange(
                    "p (i t) -> p i t", t=P)
                if spread and g % 2 == 1:
                    # emit-gating chunks: odd groups via ACT+Pool so the
                    # four normalizes finish in half the wall time
                    osb = attn_tmp.tile([P, 4, P], F32, tag="osb", bufs=1,
                                        name=f"osb{m}_{gp}")
                    nc.scalar.copy(osb[base:base + HD, :, :], o_src)
                    nc.gpsimd.tensor_mul(at_dst, osb[base:base + HD, :, :],
                                         rcp_src)
                else:
                    # normalize straight from PSUM on DVE (keeps Pool free
                    # for the next chunk's partition_all_reduce)
                    nc.vector.tensor_mul(at_dst, o_src, rcp_src)

        # ---- emit: Y^T = Wo^T @ A^T; two mo's pair up per psum bank so one
        # [P,512] copy drains them and the WAR pipeline is 4 mo's deep ----
        def emit_mb(half, mb, bank, split_tail=False, split_cols=False):
            c0 = half * 256
            wo_m = wstream.tile([P, 4, 8, P], F16, tag="wo", bufs=3,
                                name=f"wo_m{half}_{mb}")
            nc.sync.dma_start(wo_m[:], wo[:, mb * 4:(mb + 1) * 4, :, :])
            ys = ystage.tile([P, 4, 256], F16, tag="ys", name="ys")
            if split_cols:
                # token-halved matmul groups: the first half only needs the
                # first chunk of this token range normalized
                pss = [ptile([P, 2, 256], bank[pair],
                             f"ps_y{half}_{mb}_{pair}") for pair in range(2)]
                for th in range(2):
                    tsl = slice(c0 + th * P, c0 + (th + 1) * P)
                    for pair in range(2):
                        for sub2 in range(2):
                            sub = pair * 2 + sub2
                            for c in range(8):
                                nc.tensor.matmul(
                                    pss[pair][:, sub2, th * P:(th + 1) * P],
                                    wo_m[:, sub, c, :], at_sb[:, c, tsl],
                                    start=(c == 0), stop=(c == 7))
            for pair in range(2):
                ps = pss[pair] if split_cols else \
                    ptile([P, 2, 256], bank[pair], f"ps_y{half}_{mb}_{pair}")
                for sub2 in range(2):
                    sub = pair * 2 + sub2
                    if split_cols:
                        continue
                    for c in range(8):
                        nc.tensor.matmul(ps[:, sub2, :], wo_m[:, sub, c, :],
                                         at_sb[:, c, c0:c0 + 256],
                                         start=(c == 0), stop=(c == 7))
                # alternate copy engines by (mb+pair) parity so consecutive
                # same-bank drains never queue behind each other
                on_dve = (mb + pair) % 2 == 0
                ys_dst = ys[:, 2 * pair:2 * pair + 2, :] \
                    .rearrange("p i t -> p (i t)")
                ps_src = ps[:].rearrange("p i t -> p (i t)")
                if on_dve:
                    nc.vector.tensor_copy(ys_dst, ps_src)
                else:
                    nc.scalar.copy(ys_dst, ps_src)
                if split_tail:
                    nc.sync.dma_start(
                        yt_r[:, mb * 4 + 2 * pair:mb * 4 + 2 * pair + 2,
                             c0:c0 + 256],
                        ys[:, 2 * pair:2 * pair + 2, :])
            if not split_tail:
                nc.sync.dma_start(yt_r[:, mb * 4:(mb + 1) * 4, c0:c0 + 256],
                                  ys[:])

        # ---- schedule: rope chains and transposes overlap the tail of
        # phase B; attention starts the moment B's last matmul retires;
        # normalize (slack) trails the next chunk's critical ops
        k_pre(0)
        q_pre(0)
        k_pre(1)
        q_pre(1)
        attn_core(0)
        b_phase(3)
        kv_drain(3)
        k_pre(2)
        q_pre(2)
        attn_core(1)
        attn_norm(0)
        attn_norm(1)
        k_pre(3)
        q_pre(3)
        attn_core(2)
        emit_mb(0, 0, (6, 7))
        attn_core(3)
        attn_norm(2)
        emit_mb(0, 1, (6, 7))
        attn_norm(3)
        for mb in range(2, 8):
            emit_mb(0, mb, (6, 7))
        for mb in range(8):
            emit_mb(1, mb, (0, 1), split_tail=(mb == 7))

    nc.finalize()
    return nc


def host_inputs(inputs, core):
    """Build the per-core DRAM input map from full problem inputs."""
    hs = np.asarray(inputs["hidden_states"], np.float32)
    am = np.asarray(inputs["attention_mask"], np.float32)
    cos = np.asarray(inputs["cos"], np.float32)
    sin = np.asarray(inputs["sin"], np.float32)
    Wqkv = np.asarray(inputs["Wqkv"], np.float32)
    Wo = np.asarray(inputs["Wo"], np.float32)
    qw = np.asarray(inputs["q_norm_w"], np.float32)
    kw = np.asarray(inputs["k_norm_w"], np.float32)

    LS = 256
    ls = slice(core * LS, (core + 1) * LS)
    X = hs[:, ls, :].reshape(T, HID)
    xt = np.ascontiguousarray(X.T).astype(np.float16)
    cos_c = cos[:, ls, :].reshape(T, HD)
    sin_c = sin[:, ls, :].reshape(T, HD)
    sq = float(HD) ** -0.25  # sqrt(1/sqrt(HD)) = sqrt(1/8)
    swap = np.concatenate([np.arange(32, 64), np.arange(0, 32)])
    sign = np.concatenate([-np.ones(32, np.float32), np.ones(32, np.float32)])

    tabs = np.empty((T, 4, HD), np.float32)
    tabs[:, 0, :] = cos_c * qw[None, :] * sq
    tabs[:, 1, :] = sin_c * qw[swap][None, :] * sign[None, :] * sq
    tabs[:, 2, :] = cos_c * kw[None, :] * sq
    tabs[:, 3, :] = sin_c * kw[swap][None, :] * sign[None, :] * sq

    maskT = np.clip(am[0, 0, :P, :P].T, -60000.0, None).astype(np.float16)
    mask4 = np.broadcast_to(maskT[:, None, :], (P, 4, P))

    # wo[p=(par,d), mo, c, j] = Wo[h(c,par)*64+d, mo*128+j]
    woh = Wo.reshape(NH, HD, 32, P)
    wo_np = np.empty((P, 32, 8, P), np.float32)
    for par in range(2):
        for c in range(8):
            h = 8 * (c // 4) + 4 * par + (c % 4)
            wo_np[par * 64:(par + 1) * 64, :, c, :] = woh[h]
    m = {
        "xt": xt,
        "tabs": np.ascontiguousarray(tabs.reshape(T, 4 * HD)).astype(np.float16),
        "wq": np.ascontiguousarray(
            Wqkv[:, :QD].reshape(HID, NH, HD)[:, PERM, :]
            .reshape(HID, QD)).astype(np.float16),
        "wkv": np.ascontiguousarray(Wqkv[:, QD:]).astype(np.float16),
        "wo": np.ascontiguousarray(wo_np.reshape(P, 32 * 8 * P)
                                   .reshape(P, 32, 8, P)).astype(np.float16),
        "mask4": np.ascontiguousarray(mask4.reshape(P, 4 * P))
        .astype(np.float16),
    }
    return m


def assemble_output(yts):
    """yts: list of 8 [4096, 512] fp16 arrays -> [2, 2048, 4096] f32."""
    out = np.empty((2, 2048, HID), np.float32)
    for c, yt_ in enumerate(yts):
        sl = yt_.astype(np.float32).T.reshape(2, 256, HID)
        out[:, c * 256:(c + 1) * 256, :] = sl
    return out


_NC_CACHE = {}


def _get_nc():
    if "nc" not in _NC_CACHE:
        _NC_CACHE["nc"] = build_nc()
    return _NC_CACHE["nc"]


def _run(inputs, trace=False):
    from concourse.bass_utils import run_bass_kernel_spmd
    nc = _get_nc()
    in_maps = [host_inputs(inputs, c) for c in range(8)]
    res = run_bass_kernel_spmd(nc, in_maps, core_ids=list(range(8)),
                               trace=trace)
    out = assemble_output([res.results[c]["yt"] for c in range(8)])
    return out, res


def kernel(**inputs):
    out, _ = _run(inputs, trace=False)
    if not np.isfinite(out).all():
        # transient first-execution flake seen once on device; retry
        out, _ = _run(inputs, trace=False)
    return out


def _timed_runs(inputs, n=20):
    """Amortized per-execution wall time (ns) of the compiled SPMD body with
    device-resident inputs. Used by test.py; not part of the grading path."""
    import time
    import jax
    from jax.sharding import Mesh, PartitionSpec, NamedSharding
    from jax.experimental.shard_map import shard_map
    import concourse.bass2jax as b2j
    import concourse.mybir as _mb

    nc = _get_nc()
    in_maps = [host_inputs(inputs, c) for c in range(8)]
    n_cores = 8
    b2j.install_neuronx_cc_hook()
    pname = nc.partition_id_tensor.name if nc.partition_id_tensor else None
    in_names, out_names, out_avals, zero_outs = [], [], [], []
    for alloc in nc.m.functions[0].allocations:
        if not isinstance(alloc, _mb.MemoryLocationSet):
            continue
        name = alloc.memorylocations[0].name
        if alloc.kind == "ExternalInput":
            if name != pname:
                in_names.append(name)
        elif alloc.kind == "ExternalOutput":
            out_names.append(name)
            shape = tuple(alloc.tensor_shape)
            dtype = _mb.dt.np(alloc.dtype)
            out_avals.append(jax.core.ShapedArray(shape, dtype))
            zero_outs.append(np.zeros(shape, dtype))
    n_params = len(in_names)
    all_in = list(in_names) + list(out_names)
    if pname is not None:
        all_in.append(pname)

    def _body(*args):
        operands = list(args)
        if pname is not None:
            operands.append(b2j.partition_id_tensor())
        return tuple(b2j._bass_exec_p.bind(
            *operands, out_avals=tuple(out_avals), in_names=tuple(all_in),
            out_names=tuple(out_names), lowering_input_output_aliases=(),
            sim_require_finite=True, sim_require_nnan=True, nc=nc))

    devices = jax.devices()[:n_cores]
    mesh = Mesh(np.asarray(devices), ("core",))
    specs = (PartitionSpec("core"),) * (n_params + len(out_names))
    fn = jax.jit(shard_map(_body, mesh=mesh, in_specs=specs,
                           out_specs=(PartitionSpec("core"),) * len(out_names),
                           check_rep=False), keep_unused=True)
    per_core = [[np.asarray(m[nm]) for nm in in_names] for m in in_maps]
    concat_in = [np.concatenate([per_core[c][i] for c in range(n_cores)])
                 for i in range(n_params)]
    concat_zero = [np.zeros((n_cores * z.shape[0], *z.shape[1:]), z.dtype)
                   for z in zero_outs]
    sh = NamedSharding(mesh, PartitionSpec("core"))
    dev_in = [jax.device_put(a, sh) for a in concat_in + concat_zero]
    out = fn(*dev_in)
    jax.block_until_ready(out)
    best = None
    for _ in range(3):
        t0 = time.time()
        for _ in range(n):
            out = fn(*dev_in)
        jax.block_until_ready(out)
        dt = (time.time() - t0) / n * 1e9
        best = dt if best is None else min(best, dt)
    return best


# revision 51
# speedup vs baseline: 1.0102x; 1.0102x over previous
"""Trainium2 Bass kernel for nn_DFlashSelfAttention (block-sparse GQA attention).

Self-contained: builds the Bass module once, shards inputs over 8 NeuronCores
(sequence-parallel), runs via run_bass_kernel_spmd, reassembles full output.
"""

import sys as _sys
for _p in ("/opt/trn_rl_repo",):
    if _p not in _sys.path:
        _sys.path.insert(0, _p)

"""Bass/Tile kernel for DFlashSelfAttention (block-diagonal causal attention).

Sharding: sequence-parallel over L (2048 -> 8 cores x 256 positions).
Attention is block-diagonal with BLOCK=16, so positions never interact
across 16-blocks; a 256-position slice (16 blocks) is fully independent.

Per-core pipeline (T = 512 rows = 2 batches x 256 positions):
  phase A: Q = X @ Wq, one pass over Wq, all 8 psum banks.
  phase B: KV = X @ Wkv into banks freed by Q->SBUF drains.
  per 128-token chunk: fp16 RMS-norm+RoPE (rstd = exp(-ln(v)/2) keeps the
    ACT engine on one table set), PE pair-transposes of Q^T/K^T, then
    GROUP-BATCHED attention: per kv-head group g of 4 query heads one
    [128,512] psum tile holds mask+S for (head, qtok); one ACT exp; column
    sums via GPSIMD partition_all_reduce; one AV matmul lands at psum
    partitions (g%2)*64; normalization = per-pair DVE reciprocal + one DVE
    multiply straight from PSUM into A^T.
  emit: Y^T = Wo^T @ A^T per 256-token half, interleaved with the chunk-2/3
    attention chains; fp16 DRAM [4096, 512]; host transposes back.

All matmul operands fp16 (1 cy/row); psum accumulation f32; softmax in f32
via ACT exp. RMS-norm weights and the sqrt(1/8) attention scale are folded
into host-precomputed rope tables.
"""

import ml_dtypes
import numpy as np

import concourse.bass as bass
import concourse.mybir as mybir
import concourse.tile as tile
from concourse import bacc
from concourse.bass_isa import ReduceOp
from concourse.masks import make_identity

F32 = mybir.dt.float32
F16 = mybir.dt.float16

P = 128
HID = 4096
KO = HID // P          # 32 k-chunks over hidden
T = 512                # rows per core: 2 batches x 256 positions
NM = T // P            # 4 t-chunks
NH = 16
NKV = 4
HD = 64
H2 = HD // 2
QD = NH * HD           # 1024
KVD = 2 * NKV * HD     # 512 (k 256 | v 256)
EPS = 1e-6

# Q-head permutation: position p holds original head PERM[p]. Even positions
# carry heads of even kv parity groups; transposed pair-tiles then expose each
# kv group's 4 heads as one contiguous [64, 4, 128] moving operand.
PERM = [0, 4, 1, 5, 2, 6, 3, 7, 8, 12, 9, 13, 10, 14, 11, 15]


def _pin_act_tables():
    """Make every activation resolve to the natural_log_exp_and_others set so
    the act-table pass emits one load instead of ping-ponging between the
    exp-only and ln-only tables. Indices (act_func_set_id) are preserved."""
    import concourse.bacc as _bacc_mod
    from concourse import hw_specs as _hw
    real = _hw.get_activation_tables

    def pinned(arch):
        t = real(arch)
        keep = "natural_log_exp_and_others"
        if keep not in t:
            return t
        return {nm: (fns if nm == keep else set()) for nm, fns in t.items()}

    _bacc_mod.get_activation_tables = pinned
    return _bacc_mod, real


def build_nc(name="dfa"):
    _bacc_mod, _real_gat = _pin_act_tables()
    try:
        return _build_nc_inner(name)
    finally:
        _bacc_mod.get_activation_tables = _real_gat


def _build_nc_inner(name="dfa"):
    nc = bacc.Bacc(None, target_bir_lowering=False, name=name)

    xt = nc.dram_tensor("xt", [HID, T], F16, kind="ExternalInput")
    wq = nc.dram_tensor("wq", [HID, QD], F16, kind="ExternalInput")
    wkv = nc.dram_tensor("wkv", [HID, KVD], F16, kind="ExternalInput")
    wo = nc.dram_tensor("wo", [P, 32, 8, P], F16, kind="ExternalInput")
    tabs = nc.dram_tensor("tabs", [T, 4 * HD], F16, kind="ExternalInput")
    mask4 = nc.dram_tensor("mask4", [P, 4 * P], F16, kind="ExternalInput")
    yt = nc.dram_tensor("yt", [HID, T], F16, kind="ExternalOutput")

    xt_r = xt.rearrange("(ko p) t -> p ko t", p=P)
    wq_r = wq.rearrange("(kb p) n -> p kb n", p=P)
    wkv_r = wkv.rearrange("(kb p) n -> p kb n", p=P)
    tabs_r = tabs.rearrange("(m p) d -> p m d", p=P)
    yt_r = yt.rearrange("(mo p) t -> p mo t", p=P)

    from contextlib import ExitStack
    with tile.TileContext(nc) as tc, ExitStack() as ctx:
        consts = ctx.enter_context(tc.tile_pool(name="consts", bufs=1))
        xt_pool = ctx.enter_context(tc.tile_pool(name="xt", bufs=1))
        wstream = ctx.enter_context(tc.tile_pool(name="wstream", bufs=4))
        acts = ctx.enter_context(tc.tile_pool(name="acts", bufs=1))
        rope_tmp = ctx.enter_context(tc.tile_pool(name="rope_tmp", bufs=1))
        attn_tmp = ctx.enter_context(tc.tile_pool(name="attn_tmp", bufs=2))
        ystage = ctx.enter_context(tc.tile_pool(name="ystage", bufs=2))
        pp = ctx.enter_context(tc.tile_pool(name="pp", bufs=1, space="PSUM"))

        def ptile(shape, bank, name, dtype=F32):
            tot = 512 if dtype == F32 else 1024
            pad = list(shape)
            pad[-1] = max(1, tot // int(np.prod(shape[1:-1])))
            return pp.tile(shape, dtype, tag=f"b{bank}", name=name,
                           padded_shape=pad)

        # ---- leading DMAs: wq/xt fine-grained, then the rest ----
        xt_sb = xt_pool.tile([P, KO, T], F16)
        wq_fine = []
        for k in range(4):
            nc.sync.dma_start(xt_sb[:, k:k + 1, :], xt_r[:, k:k + 1, :])
            wqt = wstream.tile([P, 1, QD], F16, tag="wq1", bufs=4,
                               name=f"wq_f{k}")
            nc.sync.dma_start(wqt[:], wq_r[:, k:k + 1, :])
            wq_fine.append(wqt)

        # constants ride right after the first fine loads
        ident = consts.tile([P, P], F16)
        make_identity(nc, ident)
        eps_t = consts.tile([P, 1], F32)
        nc.vector.memset(eps_t, EPS)

        # wq coarse batches (bufs=4 so FIFO WAR stalls never starve phase A);
        # mask/tabs/wkv woven in behind the first batches
        wq_coarse = []
        wkv_tiles = []
        mask_sb = consts.tile([P, 4, P], F16)
        tabs_sb = consts.tile([P, NM, 4, HD], F16)

        def wkv_load(kb):
            wkvt = wstream.tile([P, 4, KVD], F16, tag="wkv", bufs=8,
                                name=f"wkv_k{kb}")
            nc.sync.dma_start(wkvt[:], wkv_r[:, kb * 4:(kb + 1) * 4, :])
            wkv_tiles.append(wkvt)

        for kb in range(1, 8):
            wqt = wstream.tile([P, 4, QD], F16, tag="wq4", bufs=4,
                               name=f"wq_b{kb}")
            nc.sync.dma_start(wqt[:], wq_r[:, kb * 4:(kb + 1) * 4, :])
            wq_coarse.append(wqt)
            nc.sync.dma_start(xt_sb[:, kb * 4:(kb + 1) * 4, :],
                              xt_r[:, kb * 4:(kb + 1) * 4, :])
            if kb == 2:
                nc.sync.dma_start(
                    mask_sb[:], mask4.rearrange("p (f q) -> p f q", q=P))
                nc.sync.dma_start(
                    tabs_sb[:], tabs_r.rearrange("p m (f d) -> p m f d", d=HD))
            if kb >= 3:
                wkv_load(kb - 3)
        for kb in range(5, 8):
            wkv_load(kb)

        # PE warmup: anchor the p-state ramp while the first DMAs land
        warm_src = consts.tile([P, 512], F16)
        nc.vector.memset(warm_src, 0.0)
        for w in range(3):
            wps = ptile([P, 512], 0, f"warm{w}")
            nc.tensor.matmul(wps[:], ident[:], warm_src[:],
                             start=True, stop=True)

        # ---- phase A: Q = X @ Wq (single pass, all 8 banks) ----
        ps_a = [[ptile([P, 512], 2 * m + s, f"psa{m}_{s}")
                 for s in range(2)] for m in range(NM)]

        def a_matmuls(k, wq_ap):
            for m in range(NM):
                for s in range(2):
                    nc.tensor.matmul(
                        ps_a[m][s][:], xt_sb[:, k, m * P:(m + 1) * P],
                        wq_ap[:, s * 512:(s + 1) * 512],
                        start=(k == 0), stop=(k == KO - 1))

        for k in range(4):
            a_matmuls(k, wq_fine[k][:, 0, :])
        for kb in range(1, 8):
            for kk in range(4):
                a_matmuls(kb * 4 + kk, wq_coarse[kb - 1][:, kk, :])

        # ---- Q drains (DVE/ACT split) + phase B + KV drains ----
        qn = [acts.tile([P, NH, HD], F16, tag=f"qn{m}", name=f"qn{m}")
              for m in range(NM)]

        def qn_copy(m):
            nc.vector.tensor_copy(
                qn[m][:, 0:8, :],
                ps_a[m][0][:].rearrange("p (h d) -> p h d", d=HD))
            nc.scalar.copy(
                qn[m][:, 8:16, :],
                ps_a[m][1][:].rearrange("p (h d) -> p h d", d=HD))

        ps_b = []
        kn = []
        v_sb = []

        def b_phase(m):
            pb = ptile([P, KVD], 6 if m == 3 else m, f"psb{m}")
            ps_b.append(pb)
            for kb in range(8):
                for kk in range(4):
                    k = kb * 4 + kk
                    nc.tensor.matmul(
                        pb[:], xt_sb[:, k, m * P:(m + 1) * P],
                        wkv_tiles[kb][:, kk, :],
                        start=(k == 0), stop=(k == KO - 1))

        def kv_drain(m):
            knt = acts.tile([P, NKV, HD], F16, tag=f"kn{m}", name=f"kn{m}")
            nc.vector.tensor_copy(
                knt[:], ps_b[m][:, 0:256].rearrange("p (h d) -> p h d", d=HD))
            kn.append(knt)
            vt = acts.tile([P, 256], F16, tag=f"v{m}", name=f"v{m}")
            nc.scalar.copy(vt[:], ps_b[m][:, 256:512])
            v_sb.append(vt)

        qn_copy(0)
        qn_copy(1)
        b_phase(0)
        kv_drain(0)
        qn_copy(2)
        qn_copy(3)
        b_phase(1)
        kv_drain(1)
        b_phase(2)
        kv_drain(2)

        # ---- rope helpers (fp16 math; rstd via ln/exp on ACT) ----
        def stats(src, nh, m, tag):
            sq = rope_tmp.tile([P, nh, HD], F16, tag=f"sq{nh}", bufs=1,
                               name=f"sq_{tag}")
            nc.vector.tensor_mul(sq[:], src[:], src[:])
            ssq = rope_tmp.tile([P, nh], F32, tag=f"ssq{nh}", bufs=2,
                                name=f"ssq_{tag}")
            nc.vector.reduce_sum(ssq[:], sq[:], axis=mybir.AxisListType.X)
            lnv = rope_tmp.tile([P, nh], F32, tag=f"lnv{nh}", bufs=2,
                                name=f"lnv_{tag}")
            nc.scalar.activation(lnv[:], ssq[:],
                                 mybir.ActivationFunctionType.Ln,
                                 bias=eps_t[:], scale=1.0 / HD)
            rstd = rope_tmp.tile([P, nh], F16, tag=f"rstd{nh}", bufs=2,
                                 name=f"rstd_{tag}")
            nc.scalar.activation(rstd[:], lnv[:],
                                 mybir.ActivationFunctionType.Exp, scale=-0.5)
            return rstd

        def rope_apply(src, rstd, m, nh, cf, sf, tag):
            qn2 = rope_tmp.tile([P, nh, HD], F16, tag=f"qn2_{nh}", bufs=2,
                                name=f"qn2_{tag}")
            nc.vector.tensor_mul(qn2[:], src[:],
                                 rstd[:, :, None].to_broadcast((P, nh, HD)))
            ctab = tabs_sb[:, m, cf, :]
            stab = tabs_sb[:, m, sf, :]
            o1 = rope_tmp.tile([P, nh, HD], F16, tag=f"o1_{nh}", bufs=1,
                               name=f"o1_{tag}")
            nc.vector.tensor_mul(o1[:], qn2[:],
                                 ctab[:, None, :].to_broadcast((P, nh, HD)))
            o2 = rope_tmp.tile([P, nh, HD], F16, tag=f"o2_{nh}", bufs=1,
                               name=f"o2_{tag}")
            nc.vector.tensor_mul(
                o2[:, :, 0:H2], qn2[:, :, H2:HD],
                stab[:, None, 0:H2].to_broadcast((P, nh, H2)))
            nc.vector.tensor_mul(
                o2[:, :, H2:HD], qn2[:, :, 0:H2],
                stab[:, None, H2:HD].to_broadcast((P, nh, H2)))
            out = rope_tmp.tile([P, nh * HD], F16, tag=f"ro_{nh}", bufs=2,
                                name=f"ro_{tag}")
            nc.vector.tensor_add(out[:], o1[:].rearrange("p h d -> p (h d)"),
                                 o2[:].rearrange("p h d -> p (h d)"))
            return out

        QRT_BANK = {0: 5, 1: 7, 2: 5, 3: 7}
        krT = []
        qrT = []

        def k_pre(m):
            rstd_k = stats(kn[m], NKV, m, f"k{m}")
            kr = rope_apply(kn[m], rstd_k, m, NKV, 2, 3, f"k{m}")
            kps = ptile([P, 2, P], 4, f"krT_ps{m}", F16)
            for j in range(2):
                nc.tensor.matmul(kps[:, j, :], kr[:, j * P:(j + 1) * P],
                                 ident[:], is_transpose=True)
            kt = acts.tile([P, 2, P], F16, tag=f"krT{m}", name=f"krT{m}")
            nc.vector.tensor_copy(kt[:], kps[:])
            krT.append(kt)

        def q_pre(m):
            rstd_q = stats(qn[m], NH, m, f"q{m}")
            qr = rope_apply(qn[m], rstd_q, m, NH, 0, 1, f"q{m}")
            qps = ptile([P, 8, P], QRT_BANK[m], f"qrT_ps{m}", F16)
            for j in range(8):
                nc.tensor.matmul(qps[:, j, :],
                                 qr[:, 2 * j * HD:(2 * j + 2) * HD],
                                 ident[:], is_transpose=True)
            qt = acts.tile([P, 8, P], F16, tag=f"qrT{m}", name=f"qrT{m}")
            nc.vector.tensor_copy(qt[:], qps[:])
            qrT.append(qt)

        # A^T accumulator: partition (g%2)*64+d, col c=(g//2)*4+i, token t.
        at_sb = acts.tile([P, 8, T], F16, tag="at")

        # ---- group-batched attention ----
        attn_state = {}

        def attn_core(m):
            sums = attn_tmp.tile([P, 4, 512], F32, tag="sums", bufs=1,
                                 name=f"sums{m}")
            # pair layout: partition half (g%2)*64 of column gp holds group
            # g's reciprocal, matching o_ps[gp]'s layout so one [128,512]
            # multiply normalizes both groups of a pair at once
            rcp = attn_tmp.tile([P, 2, 512], F32, tag="rcp", bufs=1,
                                name=f"rcp{m}")
            o_ps = {}
            attn_state[m] = (rcp, o_ps)
            for g in range(4):
                base = (g % 2) * HD
                gp = g // 2
                s_ps = ptile([P, 512], g % 2, f"s{m}_{g}")
                nc.tensor.matmul(s_ps[:], ident[:], mask_sb[:],
                                 start=True, stop=False)
                nc.tensor.matmul(s_ps[:],
                                 krT[m][base:base + HD, gp, :],
                                 qrT[m][base:base + HD, 4 * gp:4 * gp + 4, :],
                                 start=False, stop=True)
                est = attn_tmp.tile([P, 4, P], F16, tag="est", bufs=3,
                                    name=f"est{m}_{g}")
                nc.scalar.activation(est[:], s_ps[:],
                                     mybir.ActivationFunctionType.Exp)
                nc.gpsimd.partition_all_reduce(sums[:, g, :], est[:],
                                               channels=P,
                                               reduce_op=ReduceOp.add)
                if gp not in o_ps:
                    o_ps[gp] = ptile([P, 512], 2 + gp, f"o{m}_{gp}")
                nc.tensor.matmul(o_ps[gp][base:base + HD, :],
                                 v_sb[m][:, g * HD:(g + 1) * HD], est[:],
                                 start=True, stop=True)
                if g % 2 == 1:
                    # per-pair reciprocals into the pair layout; lane-locked
                    # halves keep the DVE queue moving
                    nc.vector.reciprocal(rcp[0:HD, gp, :],
                                         sums[0:HD, g - 1, :])
                    nc.vector.reciprocal(rcp[HD:P, gp, :],
                                         sums[HD:P, g, :])

        def attn_norm(m):
            rcp, o_ps = attn_state[m]
            for gp in range(2):
                # one multiply normalizes both groups of the pair: partition
                # halves of o_ps/rcp line up with at_sb's (g%2) layout
                nc.vector.tensor_mul(
                    at_sb[:, 4 * gp:4 * gp + 4, m * P:(m + 1) * P],
                    o_ps[gp][:].rearrange("p (i t) -> p i t", t=P),
                    rcp[:, gp, :].rearrange("p (i t) -> p i t", t=P))

        # ---- emit: Y^T = Wo^T @ A^T; two mo's pair up per psum bank so one
        # [P,512] copy drains them and the WAR pipeline is 4 mo's deep ----
        def emit_mb(half, mb, bank, split_tail=False, split_cols=False):
            c0 = half * 256
            wo_m = wstream.tile([P, 4, 8, P], F16, tag="wo", bufs=3,
                                name=f"wo_m{half}_{mb}")
            nc.sync.dma_start(wo_m[:], wo[:, mb * 4:(mb + 1) * 4, :, :])
            ys = ystage.tile([P, 4, 256], F16, tag="ys", name="ys")
            if split_cols:
                # token-halved matmul groups: the first half only needs the
                # first chunk of this token range normalized
                pss = [ptile([P, 2, 256], bank[pair],
                             f"ps_y{half}_{mb}_{pair}") for pair in range(2)]
                for th in range(2):
                    tsl = slice(c0 + th * P, c0 + (th + 1) * P)
                    for pair in range(2):
                        for sub2 in range(2):
                            sub = pair * 2 + sub2
                            for c in range(8):
                                nc.tensor.matmul(
                                    pss[pair][:, sub2, th * P:(th + 1) * P],
                                    wo_m[:, sub, c, :], at_sb[:, c, tsl],
                                    start=(c == 0), stop=(c == 7))
            for pair in range(2):
                ps = pss[pair] if split_cols else \
                    ptile([P, 2, 256], bank[pair], f"ps_y{half}_{mb}_{pair}")
                for sub2 in range(2):
                    sub = pair * 2 + sub2
                    if split_cols:
                        continue
                    for c in range(8):
                        nc.tensor.matmul(ps[:, sub2, :], wo_m[:, sub, c, :],
                                         at_sb[:, c, c0:c0 + 256],
                                         start=(c == 0), stop=(c == 7))
                # alternate copy engines by (mb+pair) parity so consecutive
                # same-bank drains never queue behind each other
                on_dve = (mb + pair) % 2 == 0
                ys_dst = ys[:, 2 * pair:2 * pair + 2, :] \
                    .rearrange("p i t -> p (i t)")
                ps_src = ps[:].rearrange("p i t -> p (i t)")
                if on_dve:
                    nc.vector.tensor_copy(ys_dst, ps_src)
                else:
                    nc.scalar.copy(ys_dst, ps_src)
                if split_tail:
                    nc.sync.dma_start(
                        yt_r[:, mb * 4 + 2 * pair:mb * 4 + 2 * pair + 2,
                             c0:c0 + 256],
                        ys[:, 2 * pair:2 * pair + 2, :])
            if not split_tail:
                nc.sync.dma_start(yt_r[:, mb * 4:(mb + 1) * 4, c0:c0 + 256],
                                  ys[:])

        # ---- schedule: rope chains and transposes overlap the tail of
        # phase B; attention starts the moment B's last matmul retires;
        # normalize (slack) trails the next chunk's critical ops
        k_pre(0)
        q_pre(0)
        k_pre(1)
        q_pre(1)
        attn_core(0)
        b_phase(3)
        kv_drain(3)
        k_pre(2)
        q_pre(2)
        attn_core(1)
        attn_norm(0)
        attn_norm(1)
        k_pre(3)
        q_pre(3)
        attn_core(2)
        emit_mb(0, 0, (6, 7))
        attn_core(3)
        attn_norm(2)
        emit_mb(0, 1, (6, 7))
        attn_norm(3)
        for mb in range(2, 8):
            emit_mb(0, mb, (6, 7))
        for mb in range(8):
            emit_mb(1, mb, (0, 1), split_tail=(mb == 7))

    nc.finalize()
    return nc


def host_inputs(inputs, core):
    """Build the per-core DRAM input map from full problem inputs."""
    hs = np.asarray(inputs["hidden_states"], np.float32)
    am = np.asarray(inputs["attention_mask"], np.float32)
    cos = np.asarray(inputs["cos"], np.float32)
    sin = np.asarray(inputs["sin"], np.float32)
    Wqkv = np.asarray(inputs["Wqkv"], np.float32)
    Wo = np.asarray(inputs["Wo"], np.float32)
    qw = np.asarray(inputs["q_norm_w"], np.float32)
    kw = np.asarray(inputs["k_norm_w"], np.float32)

    LS = 256
    ls = slice(core * LS, (core + 1) * LS)
    X = hs[:, ls, :].reshape(T, HID)
    xt = np.ascontiguousarray(X.T).astype(np.float16)
    cos_c = cos[:, ls, :].reshape(T, HD)
    sin_c = sin[:, ls, :].reshape(T, HD)
    sq = float(HD) ** -0.25  # sqrt(1/sqrt(HD)) = sqrt(1/8)
    swap = np.concatenate([np.arange(32, 64), np.arange(0, 32)])
    sign = np.concatenate([-np.ones(32, np.float32), np.ones(32, np.float32)])

    tabs = np.empty((T, 4, HD), np.float32)
    tabs[:, 0, :] = cos_c * qw[None, :] * sq
    tabs[:, 1, :] = sin_c * qw[swap][None, :] * sign[None, :] * sq
    tabs[:, 2, :] = cos_c * kw[None, :] * sq
    tabs[:, 3, :] = sin_c * kw[swap][None, :] * sign[None, :] * sq

    maskT = np.clip(am[0, 0, :P, :P].T, -60000.0, None).astype(np.float16)
    mask4 = np.broadcast_to(maskT[:, None, :], (P, 4, P))

    # wo[p=(par,d), mo, c, j] = Wo[h(c,par)*64+d, mo*128+j]
    woh = Wo.reshape(NH, HD, 32, P)
    wo_np = np.empty((P, 32, 8, P), np.float32)
    for par in range(2):
        for c in range(8):
            h = 8 * (c // 4) + 4 * par + (c % 4)
            wo_np[par * 64:(par + 1) * 64, :, c, :] = woh[h]
    m = {
        "xt": xt,
        "tabs": np.ascontiguousarray(tabs.reshape(T, 4 * HD)).astype(np.float16),
        "wq": np.ascontiguousarray(
            Wqkv[:, :QD].reshape(HID, NH, HD)[:, PERM, :]
            .reshape(HID, QD)).astype(np.float16),
        "wkv": np.ascontiguousarray(Wqkv[:, QD:]).astype(np.float16),
        "wo": np.ascontiguousarray(wo_np.reshape(P, 32 * 8 * P)
                                   .reshape(P, 32, 8, P)).astype(np.float16),
        "mask4": np.ascontiguousarray(mask4.reshape(P, 4 * P))
        .astype(np.float16),
    }
    return m


def assemble_output(yts):
    """yts: list of 8 [4096, 512] fp16 arrays -> [2, 2048, 4096] f32."""
    out = np.empty((2, 2048, HID), np.float32)
    for c, yt_ in enumerate(yts):
        sl = yt_.astype(np.float32).T.reshape(2, 256, HID)
        out[:, c * 256:(c + 1) * 256, :] = sl
    return out


_NC_CACHE = {}


def _get_nc():
    if "nc" not in _NC_CACHE:
        _NC_CACHE["nc"] = build_nc()
    return _NC_CACHE["nc"]


def _run(inputs, trace=False):
    from concourse.bass_utils import run_bass_kernel_spmd
    nc = _get_nc()
    in_maps = [host_inputs(inputs, c) for c in range(8)]
    res = run_bass_kernel_spmd(nc, in_maps, core_ids=list(range(8)),
                               trace=trace)
    out = assemble_output([res.results[c]["yt"] for c in range(8)])
    return out, res


def kernel(**inputs):
    out, _ = _run(inputs, trace=False)
    if not np.isfinite(out).all():
        # transient first-execution flake seen once on device; retry
        out, _ = _run(inputs, trace=False)
    return out


def _timed_runs(inputs, n=20):
    """Amortized per-execution wall time (ns) of the compiled SPMD body with
    device-resident inputs. Used by test.py; not part of the grading path."""
    import time
    import jax
    from jax.sharding import Mesh, PartitionSpec, NamedSharding
    from jax.experimental.shard_map import shard_map
    import concourse.bass2jax as b2j
    import concourse.mybir as _mb

    nc = _get_nc()
    in_maps = [host_inputs(inputs, c) for c in range(8)]
    n_cores = 8
    b2j.install_neuronx_cc_hook()
    pname = nc.partition_id_tensor.name if nc.partition_id_tensor else None
    in_names, out_names, out_avals, zero_outs = [], [], [], []
    for alloc in nc.m.functions[0].allocations:
        if not isinstance(alloc, _mb.MemoryLocationSet):
            continue
        name = alloc.memorylocations[0].name
        if alloc.kind == "ExternalInput":
            if name != pname:
                in_names.append(name)
        elif alloc.kind == "ExternalOutput":
            out_names.append(name)
            shape = tuple(alloc.tensor_shape)
            dtype = _mb.dt.np(alloc.dtype)
            out_avals.append(jax.core.ShapedArray(shape, dtype))
            zero_outs.append(np.zeros(shape, dtype))
    n_params = len(in_names)
    all_in = list(in_names) + list(out_names)
    if pname is not None:
        all_in.append(pname)

    def _body(*args):
        operands = list(args)
        if pname is not None:
            operands.append(b2j.partition_id_tensor())
        return tuple(b2j._bass_exec_p.bind(
            *operands, out_avals=tuple(out_avals), in_names=tuple(all_in),
            out_names=tuple(out_names), lowering_input_output_aliases=(),
            sim_require_finite=True, sim_require_nnan=True, nc=nc))

    devices = jax.devices()[:n_cores]
    mesh = Mesh(np.asarray(devices), ("core",))
    specs = (PartitionSpec("core"),) * (n_params + len(out_names))
    fn = jax.jit(shard_map(_body, mesh=mesh, in_specs=specs,
                           out_specs=(PartitionSpec("core"),) * len(out_names),
                           check_rep=False), keep_unused=True)
    per_core = [[np.asarray(m[nm]) for nm in in_names] for m in in_maps]
    concat_in = [np.concatenate([per_core[c][i] for c in range(n_cores)])
                 for i in range(n_params)]
    concat_zero = [np.zeros((n_cores * z.shape[0], *z.shape[1:]), z.dtype)
                   for z in zero_outs]
    sh = NamedSharding(mesh, PartitionSpec("core"))
    dev_in = [jax.device_put(a, sh) for a in concat_in + concat_zero]
    out = fn(*dev_in)
    jax.block_until_ready(out)
    best = None
    for _ in range(3):
        t0 = time.time()
        for _ in range(n):
            out = fn(*dev_in)
        jax.block_until_ready(out)
        dt = (time.time() - t0) / n * 1e9
        best = dt if best is None else min(best, dt)
    return best
